# revision 35
# baseline (speedup 1.0000x reference)
"""Trainium2 Bass kernel for nn_AxisNet (gnn_message_passing).

Key algebraic insight: every conv block in the reference is pointwise in the
neighbor index — the only cross-neighbor op is the final max over K in sa3.
So instead of running the MLP stack on B*N*K neighbor copies, we:
  1. run the (folded BN+ReLU) MLP once per point  -> feature table h[N, 64]
  2. gather h rows by neighbor index (dma_gather), max-reduce over K
  3. run the weight-normed FC head + axis orthonormalization per point

Sharding: batch-parallel, one batch element per NeuronCore (B == 8 cores).
All weights are tiny and replicated; everything else is per-core local, so
no collectives are needed.
"""

import numpy as np

import concourse.bacc as bacc
import concourse.mybir as mybir
import concourse.tile as tile
from concourse.masks import make_identity

F32 = mybir.dt.float32
F32R = mybir.dt.float32r
F16 = mybir.dt.float16
I16 = mybir.dt.int16

B, N, K = 8, 16384, 15
EPS = 1e-9

# float32r matmuls (1 cycle/row vs fp32's 4) were measured to give NO
# end-to-end speedup (the kernel is gather/DGE-bound, not PE-bound) while
# costing 180x accuracy (3.8e-3 vs 2.1e-5 rel err) — so plain fp32 it is.
USE_F32R = False

# The concat([xyz, feat]) @ W.T layers are computed as two accumulating
# matmuls (xyz rows of W and feat rows of W), so no partition-offset concat
# tiles are needed.  Each layer: (out_name, cout, [(src_name, lo, hi), ...])
# where (lo, hi) selects input-channel rows of the folded weight.
CONV_PLAN = [
    ("a1", 8, [("xyz", 0, 3)]),
    ("f1", 16, [("a1", 0, 8)]),
    ("f2a", 24, [("xyz", 0, 3), ("f1", 3, 19)]),
    ("f2", 32, [("f2a", 0, 24)]),
    ("f3a", 48, [("xyz", 0, 3), ("f2", 3, 35)]),
    ("h", 64, [("f3a", 0, 48)]),
]
FC_LAYERS = [(64, 32), (32, 32), (32, 6)]
D = 64  # feature table width

# weight blob column layout: one [hi-lo, cout] lhsT block per matmul piece
_PIECE_COL = {}
_c = 0
for _li, (_nm, _cout, _pieces) in enumerate(CONV_PLAN):
    for _pi, (_src, _lo, _hi) in enumerate(_pieces):
        _PIECE_COL[(_li, _pi)] = _c
        _c += _cout
for _fi, (_cin, _cout) in enumerate(FC_LAYERS):
    _PIECE_COL[(6 + _fi, 0)] = _c
    _c += _cout
W_BLOB_COLS = _c
W_BLOB_PAD = ((_c + 15) // 16) * 16


def build_module(n=N, ch=2048, name="axisnet", reps=1,
                 parts=("mlp", "gather", "fc")):
    """Build the per-core Bass module. n points, chunk size ch.

    reps > 1 repeats the whole computation sequentially inside the NEFF —
    used only for benchmarking (wall-time slope vs reps isolates HW time
    from the axon RPC overhead).
    """
    assert n % ch == 0 and ch % 128 == 0 and ch % 16 == 0
    nch = n // ch
    ng = ch // 128           # transpose groups per chunk
    half = ch // 2           # ACT granularity for the MLP
    mm = 512 if half % 512 == 0 else half   # matmul free-dim tile
    assert half % mm == 0

    nc = bacc.Bacc("TRN2", target_bir_lowering=False, name=name,
                   num_swdge_queues=4)

    xyzT = nc.dram_tensor("xyzT", [3, n], F32, kind="ExternalInput")
    idx = nc.dram_tensor("idx", [128, K, nch, ch // 16], I16, kind="ExternalInput")
    wb_d = nc.dram_tensor("wb", [128, W_BLOB_PAD], F32, kind="ExternalInput")
    bb_d = nc.dram_tensor("bb", [128, 16], F32, kind="ExternalInput")
    outb = nc.dram_tensor("outb", [128, nch, ng, 9], F32, kind="ExternalOutput")

    wb16_d = nc.dram_tensor("wb16", [128, 80], F16, kind="ExternalInput")

    with tile.TileContext(nc) as tc:
        with (
            tc.tile_pool(name="const", bufs=1) as cpool,
            tc.tile_pool(name="dram", bufs=1, space="DRAM") as dpool,
        ):
            htab = dpool.tile([n, D], F32)

            wb = cpool.tile([128, W_BLOB_PAD], F32)
            wb16 = cpool.tile([128, 80], F16)
            bb = cpool.tile([128, 16], F32)
            ident = cpool.tile([128, 128], F32)
            ident16 = cpool.tile([128, 128], F16)
            idx_t = cpool.tile([128, K, nch, ch // 16], I16)
            nc.sync.dma_start(
                out=wb[:].bitcast(F32R) if USE_F32R else wb[:],
                in_=wb_d[:].bitcast(F32R) if USE_F32R else wb_d[:])
            nc.sync.dma_start(out=wb16[:], in_=wb16_d[:])
            nc.sync.dma_start(out=bb[:], in_=bb_d[:])
            nc.sync.dma_start(out=idx_t[:], in_=idx[:])
            make_identity(nc, ident[:])
            make_identity(nc, ident16[:])

            for _rep in range(reps):
                _build_body(nc, tc, n, ch, nch, ng, half, mm,
                            xyzT, idx_t, wb, wb16, bb, ident, ident16,
                            htab, outb, parts=parts)

    nc.compile()
    return nc


def _build_body(nc, tc, n, ch, nch, ng, half, mm,
                xyzT, idx_t, wb, wb16, bb, ident, ident16, htab, outb,
                parts=("mlp", "gather", "fc")):
    if "mlp" in parts:
        if True:
            # ---------------- phase 1: per-point MLP -> feature table -------
            with (
                tc.tile_pool(name="mlp", bufs=2) as mpool,
                tc.tile_pool(name="mlp_ps", bufs=3, space="PSUM") as mpsum,
                tc.tile_pool(name="tr_ps", bufs=2, space="PSUM") as tpsum,
            ):
                for c in range(nch):
                    xyz_t = mpool.tile([3, ch], F32, tag="xyz")
                    xsrc = xyzT[:, c * ch:(c + 1) * ch]
                    nc.sync.dma_start(
                        out=(xyz_t[:].bitcast(F32R) if USE_F32R
                             else xyz_t[:]),
                        in_=xsrc.bitcast(F32R) if USE_F32R else xsrc)
                    feats = {"xyz": xyz_t}
                    for li, (nm, cout, pieces) in enumerate(CONV_PLAN):
                        dst = mpool.tile([cout, ch], F32, tag=nm)
                        feats[nm] = dst
                        for hh in range(2):
                            ps = mpsum.tile([64, half], F32, tag="mps")
                            for j in range(half // mm):
                                s = hh * half + j * mm
                                for pi, (src, lo, hi) in enumerate(pieces):
                                    wc = _PIECE_COL[(li, pi)]
                                    lhsT = wb[0:hi - lo, wc:wc + cout]
                                    rhs = feats[src][:, s:s + mm]
                                    if USE_F32R:
                                        lhsT = lhsT.bitcast(F32R)
                                        rhs = rhs.bitcast(F32R)
                                    nc.tensor.matmul(
                                        ps[0:cout, j * mm:(j + 1) * mm],
                                        lhsT=lhsT,
                                        rhs=rhs,
                                        start=(pi == 0),
                                        stop=(pi == len(pieces) - 1),
                                    )
                            dstv = dst[:, hh * half:(hh + 1) * half]
                            if USE_F32R and nm != "h":
                                dstv = dstv.bitcast(F32R)
                            nc.scalar.activation(
                                dstv,
                                ps[0:cout, :],
                                mybir.ActivationFunctionType.Relu,
                                bias=bb[0:cout, li:li + 1],
                            )
                    h = feats["h"]

                    # transpose h [64, ch] -> table rows [ch, 64]; two
                    # transposes share one psum tile -> one copy per pair
                    tst = mpool.tile([128, ng, D], F32, tag="tst")
                    for g2 in range(ng // 2):
                        tp = tpsum.tile([128, 2, D], F32, tag="tps")
                        for q in range(2):
                            g = g2 * 2 + q
                            nc.tensor.transpose(
                                tp[:, q, :], in_=h[:, g * 128:(g + 1) * 128],
                                identity=ident[0:64, 0:64],
                            )
                        nc.vector.tensor_copy(tst[:, g2 * 2:g2 * 2 + 2, :],
                                              tp[:, :, :])
                    nc.sync.dma_start(
                        out=htab[c * ch:(c + 1) * ch, :].rearrange(
                            "(g p) d -> p g d", p=128),
                        in_=tst[:],
                    )

    if not ("gather" in parts or "fc" in parts):
        return
    if True:
        if True:
            # ---------------- phase 2: gather + max + FC head + axes --------
            with (
                tc.tile_pool(name="gat", bufs=2) as gpool,
                tc.tile_pool(name="fc_ps", bufs=2, space="PSUM") as fpsum,
                tc.tile_pool(name="tr2_ps", bufs=2, space="PSUM") as t2psum,
            ):
                gq = 0  # running gather counter for queue round-robin
                for c in range(nch):
                    m = gpool.tile([128, ng, D], F32, tag="m")
                    if "gather" in parts:
                        gmax = min(ch, 2048)
                        for k in range(K):
                            g_t = gpool.tile([128, ng, D], F32,
                                             tag="g", bufs=3)
                            for hh in range(ch // gmax):
                                nc.gpsimd.dma_gather(
                                    out_ap=g_t[:, hh * (gmax // 128):
                                               (hh + 1) * (gmax // 128), :],
                                    in_ap=htab[:],
                                    idxs_ap=idx_t[:, k, c, hh * (gmax // 16):
                                                  (hh + 1) * (gmax // 16)],
                                    num_idxs=gmax,
                                    num_idxs_reg=gmax,
                                    elem_size=D,
                                    single_packet=False,
                                    queue_num=gq % 4,
                                )
                                gq += 1
                            if k == 0:
                                nc.vector.tensor_copy(m[:], g_t[:])
                            else:
                                nc.vector.tensor_tensor(
                                    out=m[:], in0=m[:], in1=g_t[:],
                                    op=mybir.AluOpType.max)
                    else:
                        # fc-only probe: fill m from htab with a plain dma
                        nc.sync.dma_start(
                            out=m[:],
                            in_=htab[c * ch:(c + 1) * ch, :].rearrange(
                                "(g p) d -> p g d", p=128))
                    if "fc" not in parts:
                        continue

                    # transpose m -> [64, ch]; two groups per psum tile
                    ht = gpool.tile([64, ch], F32, tag="ht")
                    for g2 in range(ng // 2):
                        tp = t2psum.tile([64, 2, 128], F32, tag="t2ps")
                        for q in range(2):
                            g = g2 * 2 + q
                            nc.tensor.transpose(
                                tp[:, q, :], in_=m[:, g, :],
                                identity=ident[:, :])
                        eng = nc.scalar.copy if g2 % 2 == 0 else \
                            nc.vector.tensor_copy
                        htv = ht[:, g2 * 256:(g2 + 1) * 256]
                        if USE_F32R:
                            htv = htv.bitcast(F32R)
                        eng(htv,
                            tp[:, :, :].rearrange("p a b -> p (a b)"))

                    # FC head in fp16 (halves for psum double-buffering);
                    # the last layer's output stays fp32 for the axis math
                    cur = ht
                    for fi, (cin, cout) in enumerate(FC_LAYERS):
                        wc = _PIECE_COL[(6 + fi, 0)]
                        nxt = gpool.tile([32, ch], F32, tag=f"fc{fi}")
                        for hh in range(2):
                            ps = fpsum.tile([32, half], F32, tag="fps")
                            for j in range(half // mm):
                                s = hh * half + j * mm
                                lhsT = wb[0:cin, wc:wc + cout]
                                rhs = cur[0:cin, s:s + mm]
                                if USE_F32R:
                                    lhsT = lhsT.bitcast(F32R)
                                    rhs = rhs.bitcast(F32R)
                                nc.tensor.matmul(
                                    ps[0:cout, j * mm:(j + 1) * mm],
                                    lhsT=lhsT,
                                    rhs=rhs,
                                    start=True, stop=True,
                                )
                            func = (mybir.ActivationFunctionType.Tanh
                                    if fi == 2
                                    else mybir.ActivationFunctionType.Relu)
                            nxv = nxt[0:cout, hh * half:(hh + 1) * half]
                            if USE_F32R and fi < 2:
                                nxv = nxv.bitcast(F32R)
                            nc.scalar.activation(
                                nxv,
                                ps[0:cout, :], func,
                                bias=bb[0:cout, 6 + fi:7 + fi],
                            )
                        cur = nxt

                    # transpose FC out [6, ch] -> per-point layout [128, ng, 6]
                    axt = gpool.tile([128, ng, 6], F32, tag="axt")
                    for g2 in range(ng // 2):
                        tp = t2psum.tile([128, 2, 6], F32, tag="t3ps")
                        for q in range(2):
                            g = g2 * 2 + q
                            nc.tensor.transpose(
                                tp[:, q, :], in_=cur[0:6, g * 128:(g + 1) * 128],
                                identity=ident[0:6, 0:6])
                        nc.vector.tensor_copy(axt[:, g2 * 2:g2 * 2 + 2, :],
                                              tp[:, :, :])

                    # ----- axis math (all [128, ng, *] tiles) -----
                    o9 = gpool.tile([128, ng, 9], F32, tag="o9")
                    a1_ = axt[:, :, 0:3]
                    a2_ = axt[:, :, 3:6]
                    p3 = gpool.tile([128, ng, 3], F32, tag="p3")
                    q3 = gpool.tile([128, ng, 3], F32, tag="q3")
                    s1 = gpool.tile([128, ng, 1], F32, tag="s1")
                    s2 = gpool.tile([128, ng, 1], F32, tag="s2")
                    s3 = gpool.tile([128, ng, 1], F32, tag="s3")

                    tt = nc.vector.tensor_tensor
                    red = nc.vector.tensor_reduce
                    X = mybir.AxisListType.X
                    MUL = mybir.AluOpType.mult
                    SUB = mybir.AluOpType.subtract
                    ADD = mybir.AluOpType.add

                    # s11 -> s1
                    tt(out=p3[:], in0=a1_, in1=a1_, op=MUL)
                    red(out=s1[:], in_=p3[:], axis=X, op=ADD)
                    # d12 -> s2
                    tt(out=p3[:], in0=a1_, in1=a2_, op=MUL)
                    red(out=s2[:], in_=p3[:], axis=X, op=ADD)
                    # a1n = sqrt(s11) + eps ; inv1 = 1/a1n -> s1
                    nc.scalar.sqrt(s1[:], s1[:])
                    nc.vector.tensor_scalar_add(s1[:], s1[:], EPS)
                    nc.vector.reciprocal(s1[:], s1[:])
                    # z = a1 * inv1 -> o9[:, :, 6:9]
                    tt(out=o9[:, :, 6:9], in0=a1_,
                       in1=s1[:].to_broadcast([128, ng, 3]), op=MUL)
                    # k = d12 * inv1^2 -> s2
                    tt(out=s3[:], in0=s1[:], in1=s1[:], op=MUL)
                    tt(out=s2[:], in0=s2[:], in1=s3[:], op=MUL)
                    # beta2 = a2 - k*a1 -> q3
                    tt(out=p3[:], in0=a1_,
                       in1=s2[:].to_broadcast([128, ng, 3]), op=MUL)
                    tt(out=q3[:], in0=a2_, in1=p3[:], op=SUB)
                    # nb2 = sqrt(sum q3^2) + eps ; inv2 -> s3
                    tt(out=p3[:], in0=q3[:], in1=q3[:], op=MUL)
                    red(out=s3[:], in_=p3[:], axis=X, op=ADD)
                    nc.scalar.sqrt(s3[:], s3[:])
                    nc.vector.tensor_scalar_add(s3[:], s3[:], EPS)
                    nc.vector.reciprocal(s3[:], s3[:])
                    # x = q3 * inv2 -> o9[:, :, 0:3]
                    tt(out=o9[:, :, 0:3], in0=q3[:],
                       in1=s3[:].to_broadcast([128, ng, 3]), op=MUL)
                    # y = cross(z, x) -> o9[:, :, 3:6]
                    for i in range(3):
                        zi1 = o9[:, :, 6 + (i + 1) % 3:7 + (i + 1) % 3]
                        zi2 = o9[:, :, 6 + (i + 2) % 3:7 + (i + 2) % 3]
                        xi1 = o9[:, :, (i + 1) % 3:(i + 1) % 3 + 1]
                        xi2 = o9[:, :, (i + 2) % 3:(i + 2) % 3 + 1]
                        tt(out=s1[:], in0=zi1, in1=xi2, op=MUL)
                        tt(out=s2[:], in0=zi2, in1=xi1, op=MUL)
                        tt(out=o9[:, :, 3 + i:4 + i], in0=s1[:], in1=s2[:], op=SUB)

                    nc.sync.dma_start(out=outb[:, c, :, :], in_=o9[:])


# ----------------------------------------------------------------------------
# host-side preparation
# ----------------------------------------------------------------------------

def _fold_conv(layers):
    """Fold BN-affine into conv: relu(ga*(Wx+b)+be) == relu(W'x + b')."""
    out = []
    for (w, b, ga, be) in layers:
        w = np.asarray(w, np.float32)
        b = np.asarray(b, np.float32)
        ga = np.asarray(ga, np.float32)
        be = np.asarray(be, np.float32)
        out.append((ga[:, None] * w, ga * b + be))
    return out


def _wn(fc):
    v, g, b = (np.asarray(t, np.float32) for t in fc)
    w = g[:, None] * v / np.linalg.norm(v, axis=1, keepdims=True)
    return (w, b)


def make_blobs(sa1, sa2, sa3, fc1, fc2, fc3):
    folded = _fold_conv(sa1) + _fold_conv(sa2) + _fold_conv(sa3)
    folded += [_wn(fc1), _wn(fc2), _wn(fc3)]
    wb = np.zeros((128, W_BLOB_PAD), np.float32)
    wb16 = np.zeros((128, 80), np.float16)
    bb = np.zeros((128, 16), np.float32)
    for li, (w, b) in enumerate(folded):
        bb[0:w.shape[0], li] = b
        if li < 6:
            pieces = CONV_PLAN[li][2]
        else:
            pieces = [(None, 0, w.shape[1])]
        for pi, (_, lo, hi) in enumerate(pieces):
            wc = _PIECE_COL[(li, pi)]
            wb[0:hi - lo, wc:wc + w.shape[0]] = w[:, lo:hi].T
        if li >= 6:
            wc16 = (0, 32, 64)[li - 6]
            wb16[0:w.shape[1], wc16:wc16 + w.shape[0]] = \
                w.T.astype(np.float16)
    return wb, wb16, bb


def make_idx(nb, n, ch):
    """neighbors [n, K] int -> dma_gather index layout [128, K, nch, ch//16]."""
    nch = n // ch
    arr = np.asarray(nb).astype(np.int16).reshape(nch, ch // 16, 16, K)
    out = arr.transpose(2, 3, 0, 1)          # [16 lanes, K, nch, ch//16]
    return np.ascontiguousarray(np.tile(out, (8, 1, 1, 1)))


def unpack_out(outb, n, ch):
    """outb [128, nch, ng, 9] -> [n, 9] in point order."""
    return outb.transpose(1, 2, 0, 3).reshape(n, 9)


_MODULE_CACHE = {}


def _get_module():
    key = (N, 2048)
    if key not in _MODULE_CACHE:
        _MODULE_CACHE[key] = build_module(n=N, ch=2048)
    return _MODULE_CACHE[key]


def run_on_device(inputs, trace=False):
    from concourse.bass_utils import run_bass_kernel_spmd

    nc = _get_module()
    xyz = np.asarray(inputs["xyz"], np.float32)
    nbs = np.asarray(inputs["neighbors"])
    wb, wb16, bb = make_blobs(inputs["sa1"], inputs["sa2"], inputs["sa3"],
                              inputs["fc1"], inputs["fc2"], inputs["fc3"])
    in_maps = []
    for b in range(B):
        in_maps.append({
            "xyzT": np.ascontiguousarray(xyz[b].T),
            "idx": make_idx(nbs[b], N, 2048),
            "wb": wb,
            "wb16": wb16,
            "bb": bb,
        })
    res = run_bass_kernel_spmd(nc, in_maps, core_ids=list(range(B)),
                               trace=trace)
    outs = np.stack([unpack_out(res.results[b]["outb"], N, 2048)
                     for b in range(B)])          # [B, N, 9]
    x_axis = np.ascontiguousarray(outs[:, :, 0:3])
    y_axis = np.ascontiguousarray(outs[:, :, 3:6])
    z_axis = np.ascontiguousarray(outs[:, :, 6:9])
    return (x_axis, y_axis, z_axis), res


def kernel(**inputs):
    out, _ = run_on_device(inputs, trace=False)
    return out
